# revision 1
# baseline (speedup 1.0000x reference)
"""GroupQuantLinear int4 dequant + linear on 8 Trainium2 NeuronCores.

y = x @ W^T,  W = dequant(w_packed)*w_scale + w_bias  (group size 64)

Strategy (column-parallel): shard the 12288 output rows across 8 cores
(1536 each); x replicated. Per core:
  - contraction axis K=8192 split into 64 k-tiles of 128 partitions where
    partition p == group p and k-tile k == position k within each group.
    One extra k-tile holds the per-group sums of x matched against the
    bias rows, folding the bias term (sum_g bias[o,g]*xsum[t,g]) into the
    same PSUM accumulation.
  - int4 values are host-unpacked to uint8 (still 1B/elem in HBM); the
    dequant of each k-tile is ONE DVE multiply:
        wt[128 g, O] = nib_u8[128 g, O] * sT[128 g, O]   (-> bf16)
    with sT an honest fp32 tile (no broadcast needed: partition == group).
  - matmul in bf16 (fp32 PSUM accumulation), out [128 o, 512 t] per bank;
    12 o-tiles -> 2 passes of 6 PSUM banks.
"""
import os
import sys

for _p in ("/opt/trn_rl_repo",):
    if _p not in sys.path and os.path.isdir(_p):
        sys.path.insert(0, _p)

import numpy as np
import ml_dtypes

import concourse.bacc as bacc
import concourse.mybir as mybir
import concourse.tile as tile
from concourse import bass_utils

# ---- problem constants (hardcoded per contract) ----
B, S, IN_F, OUT_F = 4, 128, 8192, 12288
GS = 64                 # quant group size
NG = IN_F // GS         # 128 groups == partitions per k-tile
N_CORES = 8
O_CORE = OUT_F // N_CORES   # 1536
T = B * S                   # 512 tokens
NK = GS + 1                 # 64 nibble k-tiles + 1 bias k-tile
N_OPASS = 2                 # PSUM-capacity passes over output tiles


def host_prep_x(x):
    """x [B,S,I] fp32 -> xt [128, NK, T] bf16 (group-partition-major)."""
    x2 = x.reshape(T, NG, GS)
    xt = np.empty((NG, NK, T), dtype=np.float32)
    xt[:, 0] = x2.sum(axis=2, dtype=np.float64).T
    xt[:, 1:] = x2.transpose(1, 2, 0)
    return xt.astype(ml_dtypes.bfloat16)


def host_prep_w(w_packed, w_scale, w_bias):
    """-> per-core (wn [2,128,64,OH] u8, sT [128,Oc] f32, bT [128,Oc] bf16).

    Nibble unpack identical to the reference: group-position q = 16*blk+4*i+j
    comes from nibble i of packed word 4*blk+j. wn is pass-major and
    partition-major so weight DMAs read long contiguous per-partition lines.
    """
    p4 = w_packed.reshape(OUT_F, NG, 4, 4)
    nibs = np.stack([(p4 >> (4 * i)) & 0xF for i in range(4)], axis=-2)
    u = nibs.reshape(OUT_F, NG, GS).astype(np.uint8)        # [O, G, 64]
    OH = O_CORE // N_OPASS
    wns, sts, bts = [], [], []
    for c in range(N_CORES):
        sl = slice(c * O_CORE, (c + 1) * O_CORE)
        uc = u[sl].transpose(1, 2, 0)                        # [128, 64, Oc]
        wn = np.empty((N_OPASS, NG, GS, OH), dtype=np.uint8)
        for p in range(N_OPASS):
            wn[p] = uc[:, :, p * OH:(p + 1) * OH]
        wns.append(wn)
        sts.append(np.ascontiguousarray(w_scale[sl, :, 0].T))        # [128,Oc] f32
        bts.append(np.ascontiguousarray(w_bias[sl, :, 0].T).astype(ml_dtypes.bfloat16))
    return wns, sts, bts


def build():
    """Build the per-core bass program (identical on all cores)."""
    NOJ = O_CORE // 128
    OPP = NOJ // N_OPASS
    OH = OPP * 128

    # ramped DMA chunk sizes: small first chunks so the PE starts early
    XCH = [1, 2, 4, 6] + [8] * 6 + [4]    # x k-tile chunks (sum 65; xsum first)
    WCH = [2, 2, 4] + [8] * 7             # weight k-tile chunks per pass (sum 64)

    nc = bacc.Bacc("TRN2", target_bir_lowering=False)
    xt_d = nc.dram_tensor("xt", [NG, NK, T], mybir.dt.bfloat16, kind="ExternalInput")
    wn_d = nc.dram_tensor("wn", [N_OPASS, NG, GS, OH], mybir.dt.uint8,
                          kind="ExternalInput")
    st_d = nc.dram_tensor("st", [NG, O_CORE], mybir.dt.float32, kind="ExternalInput")
    bt_d = nc.dram_tensor("bt", [NG, O_CORE], mybir.dt.bfloat16, kind="ExternalInput")
    yt_d = nc.dram_tensor("yt", [O_CORE, T], mybir.dt.float32,
                          kind="ExternalOutput")

    with tile.TileContext(nc) as tc:
        with (
            tc.tile_pool(name="resident", bufs=1) as rpool,
            tc.tile_pool(name="nibs", bufs=4) as bpool,
            tc.tile_pool(name="wts", bufs=6) as wpool,
            tc.tile_pool(name="psum", bufs=8, space="PSUM") as ppool,
        ):
            # bias on the vector engine's queue (feeds the opening bias
            # matmuls); scale halves on the scalar engine's queue
            bt_s = rpool.tile([NG, O_CORE], mybir.dt.bfloat16)
            nc.gpsimd.dma_start(bt_s[:, :OH], bt_d[:, :OH])
            st_s = rpool.tile([NG, O_CORE], mybir.dt.float32)
            for p in range(N_OPASS):
                nc.scalar.dma_start(st_s[:, p * OH:(p + 1) * OH],
                                    st_d[:, p * OH:(p + 1) * OH])
            # x on the gpsimd engine's queue, ramped chunks
            xt_s = rpool.tile([NG, NK, T], mybir.dt.bfloat16)
            k0 = 0
            for ch in XCH:
                nc.gpsimd.dma_start(xt_s[:, k0:k0 + ch, :], xt_d[:, k0:k0 + ch, :])
                if k0 == 0:
                    nc.gpsimd.dma_start(bt_s[:, OH:], bt_d[:, OH:])
                k0 += ch

            for p in range(N_OPASS):
                oo = p * OH
                psums = [ppool.tile([128, T], mybir.dt.float32, tag="ps",
                                    name=f"ps_{p}_{j}")
                         for j in range(OPP)]
                # bias k-tile first: needs only xsum (xt idx 0) + bt
                for j in range(OPP):
                    nc.tensor.matmul(
                        psums[j][:],
                        bt_s[:, oo + j * 128: oo + (j + 1) * 128],
                        xt_s[:, 0, :],
                        start=True, stop=False)
                k0 = 0
                for ch in WCH:
                    # weights on the sync engine's queue, chunked
                    nt = bpool.tile([NG, ch, OH], mybir.dt.uint8, tag="nib",
                                    name=f"nib_{p}_{k0}")
                    nc.sync.dma_start(nt[:], wn_d[p, :, k0:k0 + ch, :])
                    for kk in range(ch):
                        k = k0 + kk
                        wt = wpool.tile([NG, OH], mybir.dt.bfloat16, tag="wt")
                        nc.vector.tensor_mul(wt[:], nt[:, kk, :],
                                             st_s[:, oo:oo + OH])
                        for j in range(OPP):
                            nc.tensor.matmul(
                                psums[j][:],
                                wt[:, j * 128:(j + 1) * 128],
                                xt_s[:, k + 1, :],
                                start=False, stop=(k == GS - 1))
                    k0 += ch
                for j in range(OPP):
                    ot = wpool.tile([128, T], mybir.dt.float32, tag="ot")
                    nc.vector.tensor_copy(ot[:], psums[j][:])
                    nc.scalar.dma_start(
                        yt_d[oo + j * 128: oo + (j + 1) * 128, :], ot[:])

    nc.compile()
    return nc


_NC_CACHE = None


def get_nc():
    global _NC_CACHE
    if _NC_CACHE is None:
        _NC_CACHE = build()
    return _NC_CACHE


def make_in_maps(x, w_packed, w_scale, w_bias):
    xt = host_prep_x(np.asarray(x, dtype=np.float32))
    wns, sts, bts = host_prep_w(np.asarray(w_packed), np.asarray(w_scale),
                                np.asarray(w_bias))
    return [{"xt": xt, "wn": wns[c], "st": sts[c], "bt": bts[c]}
            for c in range(N_CORES)]


def assemble_out(results):
    yt = np.concatenate([np.asarray(r["yt"]) for r in results], axis=0)
    return np.ascontiguousarray(yt.T).reshape(B, S, OUT_F).astype(np.float32)


def run(x, w_packed, w_scale, w_bias, trace=False, **kw):
    nc = get_nc()
    in_maps = make_in_maps(x, w_packed, w_scale, w_bias)
    res = bass_utils.run_bass_kernel_spmd(
        nc, in_maps, core_ids=list(range(N_CORES)), trace=trace, **kw)
    return assemble_out(res.results), res


def kernel(x, w_packed, w_scale, w_bias):
    out, _ = run(x, w_packed, w_scale, w_bias, trace=False)
    return out



# revision 3
# speedup vs baseline: 1.3689x; 1.3689x over previous
"""GroupQuantLinear int4 dequant + linear on 8 Trainium2 NeuronCores.

y = x @ W^T,  W = dequant(w_packed)*w_scale + w_bias  (group size 64)

Strategy (column-parallel, fp8 DoubleRow): shard the 12288 output rows
across 8 cores (1536 each); x replicated. Per core:
  - weights are dequantized ON HOST to centered values
        wc[o,g,q] = (nib[o,g,q] - 7.5) * s[o,g]
    and shipped as fp8 e4m3 (1 B/elem).  The folded offset
        b'[o,g] = b[o,g] + 7.5*s[o,g]
    is applied through the xsum trick: one extra bf16 matmul k-tile with
    moving operand b' and stationary operand per-group sums of x.
  - contraction: partition p == group p (128 groups).  64 positions per
    group: the first M_BF positions run as bf16 matmuls (error headroom),
    the remaining 56 as 28 fp8 DoubleRow pairs (2 k-tiles per matmul,
    2 elem/cycle).
  - orientation: x is the STATIONARY operand ([128, 2, 128] token slices,
    reused across o-chunks), the weights are MOVING ([128, 2, 384]);
    output lands transposed as [token, out] tiles of [128, 384] fp32 in
    8 PSUM banks (4 token tiles x 2 o-chunks) per o-half pass (2 passes).
  - everything is SBUF-resident; DMAs are chunk-ramped across 4 queues so
    the PE starts early and never starves.
"""
import os
import sys

for _p in ("/opt/trn_rl_repo",):
    if _p not in sys.path and os.path.isdir(_p):
        sys.path.insert(0, _p)

import numpy as np
import ml_dtypes

import concourse.bacc as bacc
import concourse.mybir as mybir
import concourse.tile as tile
from concourse import bass_utils

# ---- problem constants (hardcoded per contract) ----
B, S, IN_F, OUT_F = 4, 128, 8192, 12288
GS = 64                 # quant group size
NG = IN_F // GS         # 128 groups == partitions per k-tile
N_CORES = 8
O_CORE = OUT_F // N_CORES   # 1536
T = B * S                   # 512 tokens
M_BF = 8                    # leading positions per group done in bf16
NP = (GS - M_BF) // 2       # 28 fp8 DoubleRow pairs
N_OPASS = 2                 # o-half passes
OHALF = O_CORE // N_OPASS   # 768
OCW = 384                   # PSUM tile width (2 chunks per o-half)
NT = T // 128               # 4 token tiles

F8 = ml_dtypes.float8_e4m3  # TRN fp8e4 bit-compatible (max 240, IEEE inf/nan)
BF = ml_dtypes.bfloat16


def host_prep_x(x):
    """x [B,S,I] f32 -> (xb [NG,1+M_BF,T] bf16, xf [NG,NP,2,T] e4m3)."""
    x2 = np.asarray(x, dtype=np.float32).reshape(T, NG, GS)
    xb = np.empty((NG, 1 + M_BF, T), dtype=BF)
    xb[:, 0] = x2.sum(axis=2, dtype=np.float64).T.astype(BF)
    xb[:, 1:] = x2[:, :, :M_BF].transpose(1, 2, 0).astype(BF)
    xf = np.ascontiguousarray(
        x2[:, :, M_BF:].transpose(1, 2, 0).reshape(NG, NP, 2, T)).astype(F8)
    return xb, xf


def host_prep_w(w_packed, w_scale, w_bias):
    """-> per-core (w8 [2,NG,NP,2,OHALF] e4m3, wb [2,NG,M_BF,OHALF] bf16,
                    bt [NG,O_CORE] bf16)."""
    p4 = np.asarray(w_packed).reshape(OUT_F, NG, 4, 4)
    nibs = np.stack([(p4 >> (4 * i)) & 0xF for i in range(4)], axis=-2)
    n_u = nibs.reshape(OUT_F, NG, GS).astype(np.float32)        # 0..15
    s = np.asarray(w_scale)[:, :, 0].astype(np.float32)         # [O,NG]
    b = np.asarray(w_bias)[:, :, 0].astype(np.float32)
    wc = (n_u - 7.5) * s[:, :, None]                            # centered
    bprime = (b + 7.5 * s).astype(BF)                           # [O,NG]
    w8_full = wc[:, :, M_BF:].astype(F8)                        # [O,NG,56]
    wb_full = wc[:, :, :M_BF].astype(BF)                        # [O,NG,8]
    w8s, wbs, bts = [], [], []
    for c in range(N_CORES):
        sl = slice(c * O_CORE, (c + 1) * O_CORE)
        w8 = np.ascontiguousarray(
            w8_full[sl].reshape(N_OPASS, OHALF, NG, NP, 2)
            .transpose(0, 2, 3, 4, 1))                          # [2,NG,NP,2,768]
        wb = np.ascontiguousarray(
            wb_full[sl].reshape(N_OPASS, OHALF, NG, M_BF)
            .transpose(0, 2, 3, 1))                             # [2,NG,8,768]
        bt = np.ascontiguousarray(bprime[sl].T)                 # [NG,1536]
        w8s.append(w8); wbs.append(wb); bts.append(bt)
    return w8s, wbs, bts


def build():
    nc = bacc.Bacc("TRN2", target_bir_lowering=False)
    xb_d = nc.dram_tensor("xb", [NG, 1 + M_BF, T], mybir.dt.bfloat16,
                          kind="ExternalInput")
    xf_d = nc.dram_tensor("xf", [NG, NP, 2, T], mybir.dt.float8e4,
                          kind="ExternalInput")
    w8_d = nc.dram_tensor("w8", [N_OPASS, NG, NP, 2, OHALF], mybir.dt.float8e4,
                          kind="ExternalInput")
    wb_d = nc.dram_tensor("wb", [N_OPASS, NG, M_BF, OHALF], mybir.dt.bfloat16,
                          kind="ExternalInput")
    bt_d = nc.dram_tensor("bt", [NG, O_CORE], mybir.dt.bfloat16,
                          kind="ExternalInput")
    yt_d = nc.dram_tensor("yt", [T, O_CORE], mybir.dt.float32,
                          kind="ExternalOutput")

    # DMA chunkings (ramped: small first so the PE starts early)
    XBCH = [3, 2, 4]                 # xb k-chunks (xsum + 8 bf16 tiles)
    XFCH = [4, 6, 8, 10]             # xf pair-chunks
    WBCH = [2, 2, 4]                 # wb k-chunks per pass
    W8CH = [2, 2, 4, 4, 8, 8]        # w8 pair-chunks per pass

    DR = mybir.MatmulPerfMode.DoubleRow

    with tile.TileContext(nc) as tc:
        with (
            tc.tile_pool(name="resident", bufs=1) as rpool,
            tc.tile_pool(name="outs", bufs=6) as opool,
            tc.tile_pool(name="psum", bufs=8, space="PSUM") as ppool,
        ):
            # --- resident tiles, chunk-ramped DMAs on 4 queues ---
            bt_s = rpool.tile([NG, O_CORE], mybir.dt.bfloat16)
            nc.scalar.dma_start(bt_s[:, :OHALF], bt_d[:, :OHALF])
            nc.scalar.dma_start(bt_s[:, OHALF:], bt_d[:, OHALF:])

            xb_s = rpool.tile([NG, 1 + M_BF, T], mybir.dt.bfloat16)
            k0 = 0
            for ch in XBCH:
                nc.gpsimd.dma_start(xb_s[:, k0:k0 + ch], xb_d[:, k0:k0 + ch])
                k0 += ch

            wb_s = rpool.tile([NG, N_OPASS, M_BF, OHALF], mybir.dt.bfloat16)
            for p in range(N_OPASS):
                k0 = 0
                for ch in WBCH:
                    nc.scalar.dma_start(wb_s[:, p, k0:k0 + ch],
                                        wb_d[p, :, k0:k0 + ch])
                    k0 += ch

            xf_s = rpool.tile([NG, NP, 2, T], mybir.dt.float8e4)
            i0 = 0
            for ch in XFCH:
                nc.gpsimd.dma_start(xf_s[:, i0:i0 + ch], xf_d[:, i0:i0 + ch])
                i0 += ch

            w8_s = rpool.tile([NG, N_OPASS, NP, 2, OHALF], mybir.dt.float8e4)
            for p, eng in ((0, nc.sync), (1, nc.gpsimd)):
                i0 = 0
                for ch in W8CH:
                    eng.dma_start(w8_s[:, p, i0:i0 + ch],
                                  w8_d[p, :, i0:i0 + ch])
                    i0 += ch

            # --- compute: 2 o-half passes, 8 psum banks each ---
            for p in range(N_OPASS):
                psums = [[ppool.tile([128, OCW], mybir.dt.float32, tag="ps",
                                     name=f"ps_{p}_{t}_{oc}")
                          for oc in range(2)] for t in range(NT)]
                ocol = [p * OHALF + oc * OCW for oc in range(2)]

                # bias k-tile: xsum (stationary) x b' (moving)
                for t in range(NT):
                    for oc in range(2):
                        nc.tensor.matmul(
                            psums[t][oc][:],
                            xb_s[:, 0, t * 128:(t + 1) * 128],
                            bt_s[:, ocol[oc]:ocol[oc] + OCW],
                            start=True, stop=False)

                # bf16 k-tiles
                for k in range(M_BF):
                    for t in range(NT):
                        for oc in range(2):
                            nc.tensor.matmul(
                                psums[t][oc][:],
                                xb_s[:, 1 + k, t * 128:(t + 1) * 128],
                                wb_s[:, p, k, oc * OCW:(oc + 1) * OCW],
                                start=False, stop=False)

                # fp8 DoubleRow pairs
                for i in range(NP):
                    last = i == NP - 1
                    for t in range(NT):
                        for oc in range(2):
                            nc.tensor.matmul(
                                psums[t][oc][:],
                                xf_s[:, i, :, t * 128:(t + 1) * 128],
                                w8_s[:, p, i, :, oc * OCW:(oc + 1) * OCW],
                                start=False, stop=last,
                                perf_mode=DR)

                # drain
                for t in range(NT):
                    for oc in range(2):
                        ot = opool.tile([128, OCW], mybir.dt.float32, tag="ot")
                        nc.vector.tensor_copy(ot[:], psums[t][oc][:])
                        nc.scalar.dma_start(
                            yt_d[t * 128:(t + 1) * 128,
                                 ocol[oc]:ocol[oc] + OCW],
                            ot[:])

    nc.compile()
    return nc


_NC_CACHE = None


def get_nc():
    global _NC_CACHE
    if _NC_CACHE is None:
        _NC_CACHE = build()
    return _NC_CACHE


def make_in_maps(x, w_packed, w_scale, w_bias):
    xb, xf = host_prep_x(x)
    w8s, wbs, bts = host_prep_w(w_packed, w_scale, w_bias)
    return [{"xb": xb, "xf": xf, "w8": w8s[c], "wb": wbs[c], "bt": bts[c]}
            for c in range(N_CORES)]


def assemble_out(results):
    yt = np.concatenate([np.asarray(r["yt"]) for r in results], axis=1)
    return np.ascontiguousarray(yt).reshape(B, S, OUT_F).astype(np.float32)


def run(x, w_packed, w_scale, w_bias, trace=False, **kw):
    nc = get_nc()
    in_maps = make_in_maps(x, w_packed, w_scale, w_bias)
    res = bass_utils.run_bass_kernel_spmd(
        nc, in_maps, core_ids=list(range(N_CORES)), trace=trace, **kw)
    return assemble_out(res.results), res


def kernel(x, w_packed, w_scale, w_bias):
    out, _ = run(x, w_packed, w_scale, w_bias, trace=False)
    return out


# revision 6
# speedup vs baseline: 1.5721x; 1.1485x over previous
"""GroupQuantLinear int4 dequant + linear on 8 Trainium2 NeuronCores.

y = x @ W^T,  W = dequant(w_packed)*w_scale + w_bias  (group size 64)

Strategy (column-parallel, fp8 DoubleRow): shard the 12288 output rows
across 8 cores (1536 each); x replicated. Per core:
  - weights are dequantized ON HOST to centered values
        wc[o,g,q] = (nib[o,g,q] - 7.5) * s[o,g]
    and shipped as fp8 e4m3 (1 B/elem).  The folded offset
        b'[o,g] = b[o,g] + 7.5*s[o,g]
    is applied through the xsum trick: one extra bf16 matmul k-tile with
    moving operand b' and stationary operand per-group sums of x.
  - contraction: partition p == group p (128 groups).  64 positions per
    group: the first M_BF positions run as bf16 matmuls (error headroom),
    the remaining 56 as 28 fp8 DoubleRow pairs (2 k-tiles per matmul,
    2 elem/cycle).
  - orientation: x is the STATIONARY operand ([128, 2, 128] token slices,
    reused across o-chunks), the weights are MOVING ([128, 2, 384]);
    output lands transposed as [token, out] tiles of [128, 384] fp32 in
    8 PSUM banks (4 token tiles x 2 o-chunks) per o-half pass (2 passes).
  - everything is SBUF-resident; DMAs are chunk-ramped across 4 queues so
    the PE starts early and never starves.
"""
import os
import sys

for _p in ("/opt/trn_rl_repo",):
    if _p not in sys.path and os.path.isdir(_p):
        sys.path.insert(0, _p)

import numpy as np
import ml_dtypes

import concourse.bacc as bacc
import concourse.mybir as mybir
import concourse.tile as tile
from concourse import bass_utils

# ---- problem constants (hardcoded per contract) ----
B, S, IN_F, OUT_F = 4, 128, 8192, 12288
GS = 64                 # quant group size
NG = IN_F // GS         # 128 groups == partitions per k-tile
N_CORES = 8
O_CORE = OUT_F // N_CORES   # 1536
T = B * S                   # 512 tokens
M_BF = 8                    # leading positions per group done in bf16
NP = (GS - M_BF) // 2       # 28 fp8 DoubleRow pairs
N_OPASS = 2                 # o-half passes
OHALF = O_CORE // N_OPASS   # 768
OCW = 384                   # PSUM tile width (2 chunks per o-half)
NT = T // 128               # 4 token tiles

F8 = ml_dtypes.float8_e4m3  # TRN fp8e4 bit-compatible (max 240, IEEE inf/nan)
BF = ml_dtypes.bfloat16


def host_prep_x(x):
    """x [B,S,I] f32 -> (xb [NG,1+M_BF,T] bf16, xf [NG,NP,2,T] e4m3)."""
    x2 = np.asarray(x, dtype=np.float32).reshape(T, NG, GS)
    xb = np.empty((NG, 1 + M_BF, T), dtype=BF)
    xb[:, 0] = x2.sum(axis=2, dtype=np.float64).T.astype(BF)
    xb[:, 1:] = x2[:, :, :M_BF].transpose(1, 2, 0).astype(BF)
    xf = np.ascontiguousarray(
        x2[:, :, M_BF:].transpose(1, 2, 0).reshape(NG, NP, 2, T)).astype(F8)
    return xb, xf


def host_prep_w(w_packed, w_scale, w_bias):
    """-> per-core (w8 [2,NG,NP,2,OHALF] e4m3, wb [2,NG,M_BF,OHALF] bf16,
                    bt [NG,O_CORE] bf16)."""
    p4 = np.asarray(w_packed).reshape(OUT_F, NG, 4, 4)
    nibs = np.stack([(p4 >> (4 * i)) & 0xF for i in range(4)], axis=-2)
    n_u = nibs.reshape(OUT_F, NG, GS).astype(np.float32)        # 0..15
    s = np.asarray(w_scale)[:, :, 0].astype(np.float32)         # [O,NG]
    b = np.asarray(w_bias)[:, :, 0].astype(np.float32)
    wc = (n_u - 7.5) * s[:, :, None]                            # centered
    bprime = (b + 7.5 * s).astype(BF)                           # [O,NG]
    w8_full = wc[:, :, M_BF:].astype(F8)                        # [O,NG,56]
    wb_full = wc[:, :, :M_BF].astype(BF)                        # [O,NG,8]
    w8s, wbs, bts = [], [], []
    for c in range(N_CORES):
        sl = slice(c * O_CORE, (c + 1) * O_CORE)
        w8 = np.ascontiguousarray(
            w8_full[sl].reshape(N_OPASS, OHALF, NG, NP, 2)
            .transpose(0, 2, 3, 4, 1))                          # [2,NG,NP,2,768]
        wb = np.ascontiguousarray(
            wb_full[sl].reshape(N_OPASS, OHALF, NG, M_BF)
            .transpose(0, 2, 3, 1))                             # [2,NG,8,768]
        bt = np.ascontiguousarray(bprime[sl].T)                 # [NG,1536]
        w8s.append(w8); wbs.append(wb); bts.append(bt)
    return w8s, wbs, bts


def build():
    nc = bacc.Bacc("TRN2", target_bir_lowering=False)
    xb_d = nc.dram_tensor("xb", [NG, 1 + M_BF, T], mybir.dt.bfloat16,
                          kind="ExternalInput")
    xf_d = nc.dram_tensor("xf", [NG, NP, 2, T], mybir.dt.float8e4,
                          kind="ExternalInput")
    w8_d = nc.dram_tensor("w8", [N_OPASS, NG, NP, 2, OHALF], mybir.dt.float8e4,
                          kind="ExternalInput")
    wb_d = nc.dram_tensor("wb", [N_OPASS, NG, M_BF, OHALF], mybir.dt.bfloat16,
                          kind="ExternalInput")
    bt_d = nc.dram_tensor("bt", [NG, O_CORE], mybir.dt.bfloat16,
                          kind="ExternalInput")
    yt_d = nc.dram_tensor("yt", [T, O_CORE], mybir.dt.float32,
                          kind="ExternalOutput")

    # DMA chunkings (ramped: small first so the PE starts early)
    XBCH = [3, 2, 4]                 # xb k-chunks (xsum + 8 bf16 tiles)
    XFCH = [4, 6, 8, 10]             # xf pair-chunks
    WBCH = [2, 2, 4]                 # wb k-chunks per pass
    W8CH = [2, 2, 4, 4, 8, 8]        # w8 pair-chunks per pass

    DR = mybir.MatmulPerfMode.DoubleRow

    with tile.TileContext(nc) as tc:
        with (
            tc.tile_pool(name="resident", bufs=1) as rpool,
            tc.tile_pool(name="outs", bufs=6) as opool,
            tc.tile_pool(name="psum", bufs=8, space="PSUM") as ppool,
        ):
            # --- resident tiles, chunk-ramped DMAs on 3 queues ---
            # sync: bt then w8-p0 bulk (then the 16 output DMAs at the end);
            # scalar: wb chunks (first in line -> no bf16-phase starvation);
            # gpsimd: xb, xf, then w8-p1 bulk.
            bt_s = rpool.tile([NG, O_CORE], mybir.dt.bfloat16)
            nc.sync.dma_start(bt_s[:, :OHALF], bt_d[:, :OHALF])
            nc.sync.dma_start(bt_s[:, OHALF:], bt_d[:, OHALF:])

            xb_s = rpool.tile([NG, 1 + M_BF, T], mybir.dt.bfloat16)
            k0 = 0
            for ch in XBCH:
                nc.gpsimd.dma_start(xb_s[:, k0:k0 + ch], xb_d[:, k0:k0 + ch])
                k0 += ch

            wb_s = rpool.tile([NG, N_OPASS, M_BF, OHALF], mybir.dt.bfloat16)
            for p in range(N_OPASS):
                k0 = 0
                for ch in WBCH:
                    nc.scalar.dma_start(wb_s[:, p, k0:k0 + ch],
                                        wb_d[p, :, k0:k0 + ch])
                    k0 += ch

            xf_s = rpool.tile([NG, NP, 2, T], mybir.dt.float8e4)
            i0 = 0
            for ch in XFCH:
                nc.gpsimd.dma_start(xf_s[:, i0:i0 + ch], xf_d[:, i0:i0 + ch])
                i0 += ch

            w8_s = rpool.tile([NG, N_OPASS, NP, 2, OHALF], mybir.dt.float8e4)
            for p, eng in ((0, nc.sync), (1, nc.gpsimd)):
                i0 = 0
                for ch in W8CH:
                    eng.dma_start(w8_s[:, p, i0:i0 + ch],
                                  w8_d[p, :, i0:i0 + ch])
                    i0 += ch

            # --- compute: 2 o-half passes, 8 psum banks each ---
            for p in range(N_OPASS):
                psums = [[ppool.tile([128, OCW], mybir.dt.float32, tag="ps",
                                     name=f"ps_{p}_{t}_{oc}")
                          for oc in range(2)] for t in range(NT)]
                ocol = [p * OHALF + oc * OCW for oc in range(2)]

                # bias k-tile: xsum (stationary) x b' (moving)
                for t in range(NT):
                    for oc in range(2):
                        nc.tensor.matmul(
                            psums[t][oc][:],
                            xb_s[:, 0, t * 128:(t + 1) * 128],
                            bt_s[:, ocol[oc]:ocol[oc] + OCW],
                            start=True, stop=False)

                # bf16 k-tiles
                for k in range(M_BF):
                    for t in range(NT):
                        for oc in range(2):
                            nc.tensor.matmul(
                                psums[t][oc][:],
                                xb_s[:, 1 + k, t * 128:(t + 1) * 128],
                                wb_s[:, p, k, oc * OCW:(oc + 1) * OCW],
                                start=False, stop=False)

                # fp8 DoubleRow pairs
                for i in range(NP):
                    last = i == NP - 1
                    for t in range(NT):
                        for oc in range(2):
                            nc.tensor.matmul(
                                psums[t][oc][:],
                                xf_s[:, i, :, t * 128:(t + 1) * 128],
                                w8_s[:, p, i, :, oc * OCW:(oc + 1) * OCW],
                                start=False, stop=last,
                                perf_mode=DR)

                # drain (copies split across DVE/ACT, out-DMAs on sync)
                for t in range(NT):
                    for oc in range(2):
                        ot = opool.tile([128, OCW], mybir.dt.float32, tag="ot")
                        if (t * 2 + oc) % 2 == 0:
                            nc.vector.tensor_copy(ot[:], psums[t][oc][:])
                        else:
                            nc.scalar.copy(ot[:], psums[t][oc][:])
                        nc.sync.dma_start(
                            yt_d[t * 128:(t + 1) * 128,
                                 ocol[oc]:ocol[oc] + OCW],
                            ot[:])

    nc.compile()
    return nc


_NC_CACHE = None


def get_nc():
    global _NC_CACHE
    if _NC_CACHE is None:
        _NC_CACHE = build()
    return _NC_CACHE


def make_in_maps(x, w_packed, w_scale, w_bias):
    xb, xf = host_prep_x(x)
    w8s, wbs, bts = host_prep_w(w_packed, w_scale, w_bias)
    return [{"xb": xb, "xf": xf, "w8": w8s[c], "wb": wbs[c], "bt": bts[c]}
            for c in range(N_CORES)]


def assemble_out(results):
    yt = np.concatenate([np.asarray(r["yt"]) for r in results], axis=1)
    return np.ascontiguousarray(yt).reshape(B, S, OUT_F).astype(np.float32)


def run(x, w_packed, w_scale, w_bias, trace=False, **kw):
    nc = get_nc()
    in_maps = make_in_maps(x, w_packed, w_scale, w_bias)
    res = bass_utils.run_bass_kernel_spmd(
        nc, in_maps, core_ids=list(range(N_CORES)), trace=trace, **kw)
    return assemble_out(res.results), res


def kernel(x, w_packed, w_scale, w_bias):
    out, _ = run(x, w_packed, w_scale, w_bias, trace=False)
    return out
